# revision 34
# baseline (speedup 1.0000x reference)
"""Multi-head attention (B=2, L=4096, C=512, H=8, Dh=64) on 8 trn2 cores.

Sharding: data-parallel over batch (4 cores per batch element) x
tensor-parallel over heads (2 heads per core). Softmax normalization and
the head-sum happen ON DEVICE; each core returns its [L, C] contribution
and the host adds the four column-block contributions per batch + bias.

The PE is bound by its single PSUM write port (1 output column/cycle,
~853ns per k-tile of attention), so every matmul stays in the (128,128)
tile mode and everything else hides behind that port-rate floor:
  - scores: K=128 matmuls against zero-padded kT (kTz[:, h] has the other
    head's 64 rows zeroed) -- same port cost as K=64, no PE mode switch.
  - V per (k-tile, head) as [V_h(64) | ones(1)]: the AV matmul emits the
    softmax denominator in output row 64 for free.
  - exp: per-head 1-bank score tiles; ScalarE true-exp serves head0 and
    VectorE Schraudolph (y_bits = int16(x*A+B) viewed as bf16; the
    denominator cancels the common-mode error) serves head1, each into its
    own pt tile -- both engines ~80% busy, no cross-engine false deps, and
    each score bank frees ~1.1us after its scores land (inside the ring-2
    reuse window).
  - AV lags the scores by 2 k-tiles so exp latency and head-of-line copy
    delays on the exp engines never stall the PE.
  - drains normalize on device: VectorE reciprocal of the denominator row,
    GpSimd partition_broadcast, then one scalar_tensor_tensor per head
    writes normalized attention into a single concatenated [128, L] tile
    (head0 rows 0:64, head1 rows 64:128, via partition-offset write).
  - out-proj is then ONE K=128 matmul per q-tile (attn_cat.T @ wo_cat =
    head0+head1 contributions summed in PSUM): half the port cost and half
    the output DMA of per-head projection, interleaved into the next
    q-chunk so the store stream spreads across the whole attention phase.
  - V-projection runs inside qc0's attention loop: its 4x128-col
    LDWEIGHTS per 128 output columns hide in the attention stream's spare
    weight-port slots instead of bounding a separate phase.
  - PSUM: 2x per-head score banks (4) + 2 AV accumulators + 1 out-proj +
    1 V bank = 8.
  - x is host-packed [p, g(8), c(4), 512] so each of 8 x-DMAs moves 4KB
    contiguous per-partition lines (1KB lines halve DMA throughput).
"""

import ml_dtypes
import numpy as np

B, L, C, H = 2, 4096, 512, 8
DH = C // H  # 64
P = 128
NCORES = 8
HEADS_PER_CORE = 2
CORES_PER_BATCH = 4

QCHUNK = 512  # q columns per attention block (1 PSUM bank per head)
NQC = L // QCHUNK  # 8
NKT = L // P  # 32 k-tiles
NCC = C // P  # 4 contraction chunks for projections
NG = 8  # x-load groups (512 tokens each)

# Schraudolph bf16 exp: bits = int16(x * SCH_A + SCH_B); view as bf16
SCH_A = 128 * 1.4426950408889634  # 128 * log2(e)
SCH_B = 16248.5

_cached = {}


def _build(reps=1):
    import concourse.mybir as mybir
    import concourse.tile as tile
    from concourse import bacc

    F32 = mybir.dt.float32
    BF16 = mybir.dt.bfloat16
    I16 = mybir.dt.int16
    EXP = mybir.ActivationFunctionType.Exp
    MULT = mybir.AluOpType.mult

    nc = bacc.Bacc("TRN2", target_bir_lowering=False, debug=False,
                   num_devices=NCORES)
    # x host-packed [p, g, c, n]: one DMA per g moves contiguous
    # 4KB-per-partition lines and delivers all 4 contraction chunks for a
    # 512-token slice
    xT = nc.dram_tensor("xT", [P, NG, NCC, L // NG], BF16,
                        kind="ExternalInput").ap()
    wq = nc.dram_tensor("wq", [P, NCC, P], BF16, kind="ExternalInput").ap()
    wk = nc.dram_tensor("wk", [P, NCC, P], BF16, kind="ExternalInput").ap()
    wv = nc.dram_tensor("wv", [P, NCC, P], BF16, kind="ExternalInput").ap()
    wo = nc.dram_tensor("wo", [P, C], BF16, kind="ExternalInput").ap()
    # partition-major [p, qtile, c]: 4KB-per-partition contiguous stores
    out = nc.dram_tensor("out", [P, L // P, C], BF16,
                         kind="ExternalOutput").ap()

    with tile.TileContext(nc) as tc:
        import contextlib
        loop_cm = tc.For_i(0, reps, 1) if reps > 1 else contextlib.nullcontext()
        with (
            tc.tile_pool(name="persist", bufs=1) as persist,
            tc.tile_pool(name="xpool", bufs=1) as xpool,
            tc.tile_pool(name="ptp", bufs=6) as ptp,
            tc.tile_pool(name="small", bufs=2) as small,
            tc.tile_pool(name="outp", bufs=4) as outp,
            loop_cm,
        ):
            # preload the exp table set so the first real exp doesn't pay
            # the ~2.7us ACT_TABLE_LOAD mid-pipeline
            warm_in = small.tile([1, 8], F32, tag="warm_in")
            warm_out = small.tile([1, 8], F32, tag="warm_out")
            nc.vector.memset(warm_in, 0.0)
            nc.scalar.activation(warm_out, warm_in, EXP)

            # ---- load inputs ----
            wq_t = persist.tile([P, NCC, P], BF16)
            wk_t = persist.tile([P, NCC, P], BF16)
            wv_t = persist.tile([P, NCC, P], BF16)
            # wo_cat [128, C]: head0 rows 0:64, head1 rows 64:128 -- the
            # natural layout for the concatenated K=128 out-proj
            wo_t = persist.tile([P, C], BF16)
            # dma order follows first-use: wk + x[0] gate the first matmul.
            # x-loads fan out over the engines' own DMA queues (engines are
            # idle here): one queue's descriptor stream bursts at ~half the
            # aggregate DMA rate, so parallel queues ~double x throughput
            xt = xpool.tile([P, NG, NCC, L // NG], BF16)
            nc.sync.dma_start(wk_t, wk)
            nc.sync.dma_start(xt[:, 0], xT[:, 0])
            nc.sync.dma_start(wq_t, wq)
            nc.sync.dma_start(wv_t, wv)
            for g in range(1, NG):
                nc.sync.dma_start(xt[:, g], xT[:, g])
            nc.sync.dma_start(wo_t, wo)

            qT = persist.tile([P, L], BF16)
            # K^T zero-padded per head: kTz[:, h, :] has rows outside
            # [h*64, (h+1)*64) zeroed, so scores run as K=128 matmuls in
            # the same 128x128 tiling mode as everything else
            kTz = persist.tile([P, HEADS_PER_CORE, L], BF16)
            # per (k-tile, head): [V_h (64) | ones (1)]; the ones column
            # makes the AV matmul emit the softmax denominator in row 64
            v_store = persist.tile([P, NKT, HEADS_PER_CORE, DH + 1], BF16)
            # normalized attention, concatenated: h0 rows 0:64, h1 64:128
            attn = persist.tile([P, L], BF16)

            nc.gpsimd.memset(v_store[:, :, :, DH], 1.0)
            nc.gpsimd.memset(kTz, 0.0)

            # ---- Q/K projections (V is emitted inside qc0's attention
            # loop below: its 4x128-col LDWEIGHTS per 128 output columns
            # would otherwise bound this phase on the weight port) ----
            with tc.tile_pool(name="pj_ps", bufs=2, space="PSUM") as pj_ps:
                for j in range(NG):
                    sl = slice(j * 512, (j + 1) * 512)
                    ps = pj_ps.tile([P, 512], F32, tag="qk_ps")
                    for c in range(NCC):
                        nc.tensor.matmul(
                            ps, wk_t[:, c, :], xt[:, j, c, :],
                            start=(c == 0), stop=(c == NCC - 1),
                        )
                    nc.scalar.copy(kTz[0:DH, 0, sl], ps[0:DH, :])
                    nc.vector.tensor_copy(kTz[DH:P, 1, sl], ps[DH:P, :])
                    ps = pj_ps.tile([P, 512], F32, tag="qk_ps")
                    for c in range(NCC):
                        nc.tensor.matmul(
                            ps, wq_t[:, c, :], xt[:, j, c, :],
                            start=(c == 0), stop=(c == NCC - 1),
                        )
                    if j % 2 == 0:
                        nc.vector.tensor_copy(qT[:, sl], ps)
                    else:
                        nc.scalar.copy(qT[:, sl], ps)

            # ---- attention + interleaved V-proj and out-proj ----
            s_ps_cm = tc.tile_pool(name="s_ps", bufs=2, space="PSUM")
            a_ps_cm = tc.tile_pool(name="a_ps", bufs=2, space="PSUM")
            o_ps_cm = tc.tile_pool(name="o_ps", bufs=1, space="PSUM")
            v_ps_cm = tc.tile_pool(name="v_ps", bufs=1, space="PSUM")
            s_ps = s_ps_cm.__enter__()
            a_ps = a_ps_cm.__enter__()
            o_ps = o_ps_cm.__enter__()
            v_ps = v_ps_cm.__enter__()

            def emit_vproj(r):
                # V k-tile r: [tokens, 128] = x-chunk.T @ wv; its LDWs
                # hide in the attention stream's spare weight-port slots
                g, ri = r // 4, r % 4
                rs = slice(ri * P, (ri + 1) * P)
                ps = v_ps.tile([P, P], F32, tag="v_ps")
                for c in range(NCC):
                    nc.tensor.matmul(
                        ps, xt[:, g, c, rs], wv_t[:, c, :],
                        start=(c == 0), stop=(c == NCC - 1),
                    )
                # both heads in one strided copy [128, 2, 64]
                if r % 2 == 0:
                    nc.vector.tensor_copy(
                        v_store[:, r, :, 0:DH],
                        ps.rearrange("p (h d) -> p h d", h=2))
                else:
                    nc.scalar.copy(
                        v_store[:, r, :, 0:DH],
                        ps.rearrange("p (h d) -> p h d", h=2))

            def emit_av(att, okt, opt):
                for h in range(HEADS_PER_CORE):
                    nc.tensor.matmul(
                        att[h],
                        v_store[:, okt, h, :],
                        opt[h],
                        start=(okt == 0), stop=(okt == NKT - 1),
                    )

            def emit_oproj(qc, pool):
                # concatenated out-proj: one K=128 matmul per q-tile sums
                # both heads' normalized contributions in PSUM
                osb = outp.tile([P, 4, C], BF16, tag="osb", name="osb")
                for i in range(4):
                    qt = 4 * qc + i
                    ps = pool.tile([P, C], F32, tag="o_ps")
                    nc.tensor.matmul(
                        ps, attn[:, qt * P:(qt + 1) * P],
                        wo_t, start=True, stop=True)
                    if i % 2 == 0:
                        nc.scalar.copy(osb[:, i, :], ps)
                    else:
                        nc.vector.tensor_copy(osb[:, i, :], ps)
                nc.sync.dma_start(out[:, 4 * qc:4 * qc + 4, :], osb)

            def emit_drain(qc, att, h):
                # normalize on device: stage the denominator row to SBUF
                # (ScalarE), approx reciprocal (~18 bits; dens are
                # well-conditioned positive sums -- the custom-DVE op needs
                # SBUF input), fan out across 64 partitions (GpSimd), then
                # one STT into the concatenated attn tile
                qsl = slice(qc * QCHUNK, (qc + 1) * QCHUNK)
                rw = small.tile([1, QCHUNK], F32, tag=f"rw{h}",
                                name=f"rw{h}")
                rd = small.tile([1, QCHUNK], F32, tag=f"rd{h}",
                                name=f"rd{h}")
                rb = small.tile([DH, QCHUNK], F32, tag=f"rb{h}",
                                name=f"rb{h}")
                nc.scalar.copy(rw, att[h][DH:DH + 1, :])
                nc.vector.reciprocal_approx_fast(rd, rw)
                nc.gpsimd.partition_broadcast(rb, rd)
                nc.vector.scalar_tensor_tensor(
                    attn[h * DH:(h + 1) * DH, qsl],
                    att[h][0:DH, :], 1.0, rb, MULT, MULT)

            prev = None  # att tiles of the previous q-chunk
            for qc in range(NQC):
                qsl = slice(qc * QCHUNK, (qc + 1) * QCHUNK)
                att = [a_ps.tile([DH + 1, QCHUNK], F32, tag="att",
                                 name=f"att{_h}")
                       for _h in range(HEADS_PER_CORE)]
                pend = []  # (kt, (pt0, pt1)) waiting for their AV matmuls
                for kt in range(NKT):
                    if qc == 0:
                        emit_vproj(kt)
                    sps_h = [s_ps.tile([P, QCHUNK], F32, tag=f"sh{_h}",
                                       name=f"sps{_h}")
                             for _h in range(HEADS_PER_CORE)]
                    for h in range(HEADS_PER_CORE):
                        nc.tensor.matmul(
                            sps_h[h],
                            kTz[:, h, kt * P:(kt + 1) * P],
                            qT[:, qsl],
                            start=True, stop=True,
                        )
                    pt0 = ptp.tile([P, QCHUNK], BF16, tag="pt0")
                    pt1 = ptp.tile([P, QCHUNK], BF16, tag="pt1")
                    nc.scalar.activation(pt0, sps_h[0], EXP)
                    nc.vector.tensor_scalar(
                        pt1.bitcast(I16), sps_h[1], SCH_A, SCH_B, MULT,
                        mybir.AluOpType.add)
                    pend.append((kt, (pt0, pt1)))
                    if len(pend) > 3:
                        emit_av(att, *pend.pop(0))
                    if qc > 0 and kt == 1:
                        emit_drain(qc - 1, prev, 0)
                    if qc > 0 and kt == 6:
                        emit_drain(qc - 1, prev, 1)
                    if qc > 0 and kt == 9:
                        emit_oproj(qc - 1, o_ps)
                for p_ in pend:
                    emit_av(att, *p_)
                prev = att
            emit_drain(NQC - 1, prev, 0)
            emit_drain(NQC - 1, prev, 1)
            v_ps_cm.__exit__(None, None, None)
            o_ps_cm.__exit__(None, None, None)
            a_ps_cm.__exit__(None, None, None)
            s_ps_cm.__exit__(None, None, None)

            # final q-chunk's out-proj with a deep PSUM ring (all banks
            # free now) so its 4 matmuls don't serialize on the copies
            with tc.tile_pool(name="f_ps", bufs=4, space="PSUM") as f_ps:
                emit_oproj(NQC - 1, f_ps)

    nc.compile()
    return nc


def _get_nc(reps=1):
    key = f"nc{reps}"
    if key not in _cached:
        _cached[key] = _build(reps)
    return _cached[key]


def _build_in_maps(inputs):
    x = np.asarray(inputs["x"], dtype=np.float32)
    Wq = np.asarray(inputs["Wq"], dtype=np.float32)
    Wk = np.asarray(inputs["Wk"], dtype=np.float32)
    Wv = np.asarray(inputs["Wv"], dtype=np.float32)
    Wo = np.asarray(inputs["Wo"], dtype=np.float32)

    scale = np.float32(1.0 / np.sqrt(DH))
    in_maps = []
    for core in range(NCORES):
        b = core // CORES_PER_BATCH
        j = core % CORES_PER_BATCH
        csl = slice(j * P, (j + 1) * P)
        bf = ml_dtypes.bfloat16
        # xT [p, g, c, n_inner]: x[b].T is [C, L]; block C into (c, p) and
        # L into (g, n)
        xTb = (x[b].T.astype(bf)
               .reshape(NCC, P, NG, L // NG).transpose(1, 2, 0, 3))
        in_maps.append({
            "xT": np.ascontiguousarray(xTb),
            "wq": np.ascontiguousarray((Wq[:, csl] * scale).astype(bf)
                                       .reshape(NCC, P, P).transpose(1, 0, 2)),
            "wk": np.ascontiguousarray(Wk[:, csl].astype(bf)
                                       .reshape(NCC, P, P).transpose(1, 0, 2)),
            "wv": np.ascontiguousarray(Wv[:, csl].astype(bf)
                                       .reshape(NCC, P, P).transpose(1, 0, 2)),
            "wo": np.ascontiguousarray(Wo[csl, :].astype(bf)),
        })
    return in_maps


def kernel(x, Wq, Wk, Wv, Wo, bo):
    from concourse import bass_utils

    bo = np.asarray(bo, dtype=np.float32)
    in_maps = _build_in_maps(
        {"x": x, "Wq": Wq, "Wk": Wk, "Wv": Wv, "Wo": Wo})

    res = bass_utils.run_bass_kernel_spmd(
        _get_nc(), in_maps, core_ids=list(range(NCORES)))

    out = np.zeros((B, L, C), dtype=np.float32)
    for core in range(NCORES):
        r = res.results[core]
        b = core // CORES_PER_BATCH
        o = np.asarray(r["out"]).astype(np.float32)
        out[b] += o.transpose(1, 0, 2).reshape(L, C)
    out += bo[None, None, :]
    return out


# revision 35
# speedup vs baseline: 1.0058x; 1.0058x over previous
"""Multi-head attention (B=2, L=4096, C=512, H=8, Dh=64) on 8 trn2 cores.

Sharding: data-parallel over batch (4 cores per batch element) x
tensor-parallel over heads (2 heads per core). Softmax normalization and
the head-sum happen ON DEVICE; each core returns its [L, C] contribution
and the host adds the four column-block contributions per batch + bias.

The PE is bound by its single PSUM write port (1 output column/cycle,
~853ns per k-tile of attention), so every matmul stays in the (128,128)
tile mode and everything else hides behind that port-rate floor:
  - scores: K=128 matmuls against zero-padded kT (kTz[:, h] has the other
    head's 64 rows zeroed) -- same port cost as K=64, no PE mode switch.
  - V per (k-tile, head) as [V_h(64) | ones(1)]: the AV matmul emits the
    softmax denominator in output row 64 for free.
  - exp: per-head 1-bank score tiles; ScalarE true-exp serves head0 and
    VectorE Schraudolph (y_bits = int16(x*A+B) viewed as bf16; the
    denominator cancels the common-mode error) serves head1, each into its
    own pt tile -- both engines ~80% busy, no cross-engine false deps, and
    each score bank frees ~1.1us after its scores land (inside the ring-2
    reuse window).
  - AV lags the scores by 2 k-tiles so exp latency and head-of-line copy
    delays on the exp engines never stall the PE.
  - drains normalize on device: VectorE reciprocal of the denominator row,
    GpSimd partition_broadcast, then one scalar_tensor_tensor per head
    writes normalized attention into a single concatenated [128, L] tile
    (head0 rows 0:64, head1 rows 64:128, via partition-offset write).
  - out-proj is then ONE K=128 matmul per q-tile (attn_cat.T @ wo_cat =
    head0+head1 contributions summed in PSUM): half the port cost and half
    the output DMA of per-head projection, interleaved into the next
    q-chunk so the store stream spreads across the whole attention phase.
  - V-projection runs inside qc0's attention loop: its 4x128-col
    LDWEIGHTS per 128 output columns hide in the attention stream's spare
    weight-port slots instead of bounding a separate phase.
  - PSUM: 2x per-head score banks (4) + 2 AV accumulators + 1 out-proj +
    1 V bank = 8.
  - x is host-packed [p, g(8), c(4), 512] so each of 8 x-DMAs moves 4KB
    contiguous per-partition lines (1KB lines halve DMA throughput).
"""

import ml_dtypes
import numpy as np

B, L, C, H = 2, 4096, 512, 8
DH = C // H  # 64
P = 128
NCORES = 8
HEADS_PER_CORE = 2
CORES_PER_BATCH = 4

QCHUNK = 512  # q columns per attention block (1 PSUM bank per head)
NQC = L // QCHUNK  # 8
NKT = L // P  # 32 k-tiles
NCC = C // P  # 4 contraction chunks for projections
NG = 8  # x-load groups (512 tokens each)

# Schraudolph bf16 exp: bits = int16(x * SCH_A + SCH_B); view as bf16
SCH_A = 128 * 1.4426950408889634  # 128 * log2(e)
SCH_B = 16248.5

_cached = {}


def _build(reps=1):
    import concourse.mybir as mybir
    import concourse.tile as tile
    from concourse import bacc

    F32 = mybir.dt.float32
    BF16 = mybir.dt.bfloat16
    I16 = mybir.dt.int16
    EXP = mybir.ActivationFunctionType.Exp
    MULT = mybir.AluOpType.mult

    nc = bacc.Bacc("TRN2", target_bir_lowering=False, debug=False,
                   num_devices=NCORES)
    # x host-packed [p, g, c, n]: one DMA per g moves contiguous
    # 4KB-per-partition lines and delivers all 4 contraction chunks for a
    # 512-token slice
    xT = nc.dram_tensor("xT", [P, NG, NCC, L // NG], BF16,
                        kind="ExternalInput").ap()
    wq = nc.dram_tensor("wq", [P, NCC, P], BF16, kind="ExternalInput").ap()
    wk = nc.dram_tensor("wk", [P, NCC, P], BF16, kind="ExternalInput").ap()
    wv = nc.dram_tensor("wv", [P, NCC, P], BF16, kind="ExternalInput").ap()
    wo = nc.dram_tensor("wo", [P, C], BF16, kind="ExternalInput").ap()
    # partition-major [p, qtile, c]: 4KB-per-partition contiguous stores
    out = nc.dram_tensor("out", [P, L // P, C], BF16,
                         kind="ExternalOutput").ap()

    with tile.TileContext(nc) as tc:
        import contextlib
        loop_cm = tc.For_i(0, reps, 1) if reps > 1 else contextlib.nullcontext()
        with (
            tc.tile_pool(name="persist", bufs=1) as persist,
            tc.tile_pool(name="xpool", bufs=1) as xpool,
            tc.tile_pool(name="ptp", bufs=6) as ptp,
            tc.tile_pool(name="small", bufs=2) as small,
            tc.tile_pool(name="outp", bufs=4) as outp,
            loop_cm,
        ):
            # preload the exp table set so the first real exp doesn't pay
            # the ~2.7us ACT_TABLE_LOAD mid-pipeline
            warm_in = small.tile([1, 8], F32, tag="warm_in")
            warm_out = small.tile([1, 8], F32, tag="warm_out")
            nc.vector.memset(warm_in, 0.0)
            nc.scalar.activation(warm_out, warm_in, EXP)

            # ---- load inputs ----
            wq_t = persist.tile([P, NCC, P], BF16)
            wk_t = persist.tile([P, NCC, P], BF16)
            wv_t = persist.tile([P, NCC, P], BF16)
            # wo_cat [128, C]: head0 rows 0:64, head1 rows 64:128 -- the
            # natural layout for the concatenated K=128 out-proj
            wo_t = persist.tile([P, C], BF16)
            # dma order follows first-use: wk + x[0] gate the first matmul.
            # x-loads fan out over the engines' own DMA queues (engines are
            # idle here): one queue's descriptor stream bursts at ~half the
            # aggregate DMA rate, so parallel queues ~double x throughput
            xt = xpool.tile([P, NG, NCC, L // NG], BF16)
            nc.sync.dma_start(wk_t, wk)
            nc.sync.dma_start(xt[:, 0], xT[:, 0])
            nc.sync.dma_start(wq_t, wq)
            nc.sync.dma_start(wv_t, wv)
            for g in range(1, NG):
                nc.sync.dma_start(xt[:, g], xT[:, g])
            nc.sync.dma_start(wo_t, wo)

            qT = persist.tile([P, L], BF16)
            # K^T zero-padded per head: kTz[:, h, :] has rows outside
            # [h*64, (h+1)*64) zeroed, so scores run as K=128 matmuls in
            # the same 128x128 tiling mode as everything else
            kTz = persist.tile([P, HEADS_PER_CORE, L], BF16)
            # per (k-tile, head): [V_h (64) | ones (1)]; the ones column
            # makes the AV matmul emit the softmax denominator in row 64
            v_store = persist.tile([P, NKT, HEADS_PER_CORE, DH + 1], BF16)
            # normalized attention, concatenated: h0 rows 0:64, h1 64:128
            attn = persist.tile([P, L], BF16)

            nc.gpsimd.memset(v_store[:, :, :, DH], 1.0)
            nc.gpsimd.memset(kTz, 0.0)

            # ---- Q/K projections (V is emitted inside qc0's attention
            # loop below: its 4x128-col LDWEIGHTS per 128 output columns
            # would otherwise bound this phase on the weight port) ----
            with tc.tile_pool(name="pj_ps", bufs=2, space="PSUM") as pj_ps:
                for j in range(NG):
                    sl = slice(j * 512, (j + 1) * 512)
                    ps = pj_ps.tile([P, 512], F32, tag="qk_ps")
                    for c in range(NCC):
                        nc.tensor.matmul(
                            ps, wk_t[:, c, :], xt[:, j, c, :],
                            start=(c == 0), stop=(c == NCC - 1),
                        )
                    nc.scalar.copy(kTz[0:DH, 0, sl], ps[0:DH, :])
                    nc.vector.tensor_copy(kTz[DH:P, 1, sl], ps[DH:P, :])
                    ps = pj_ps.tile([P, 512], F32, tag="qk_ps")
                    for c in range(NCC):
                        nc.tensor.matmul(
                            ps, wq_t[:, c, :], xt[:, j, c, :],
                            start=(c == 0), stop=(c == NCC - 1),
                        )
                    if j % 2 == 0:
                        nc.vector.tensor_copy(qT[:, sl], ps)
                    else:
                        nc.scalar.copy(qT[:, sl], ps)

            # ---- attention + interleaved V-proj and out-proj ----
            s_ps_cm = tc.tile_pool(name="s_ps", bufs=2, space="PSUM")
            a_ps_cm = tc.tile_pool(name="a_ps", bufs=2, space="PSUM")
            o_ps_cm = tc.tile_pool(name="o_ps", bufs=1, space="PSUM")
            v_ps_cm = tc.tile_pool(name="v_ps", bufs=1, space="PSUM")
            s_ps = s_ps_cm.__enter__()
            a_ps = a_ps_cm.__enter__()
            o_ps = o_ps_cm.__enter__()
            v_ps = v_ps_cm.__enter__()

            def emit_vproj(r):
                # V k-tile r: [tokens, 128] = x-chunk.T @ wv; its LDWs
                # hide in the attention stream's spare weight-port slots
                g, ri = r // 4, r % 4
                rs = slice(ri * P, (ri + 1) * P)
                ps = v_ps.tile([P, P], F32, tag="v_ps")
                for c in range(NCC):
                    nc.tensor.matmul(
                        ps, xt[:, g, c, rs], wv_t[:, c, :],
                        start=(c == 0), stop=(c == NCC - 1),
                    )
                # both heads in one strided copy [128, 2, 64]
                if r % 2 == 0:
                    nc.vector.tensor_copy(
                        v_store[:, r, :, 0:DH],
                        ps.rearrange("p (h d) -> p h d", h=2))
                else:
                    nc.scalar.copy(
                        v_store[:, r, :, 0:DH],
                        ps.rearrange("p (h d) -> p h d", h=2))

            def emit_av(att, okt, opt):
                for h in range(HEADS_PER_CORE):
                    nc.tensor.matmul(
                        att[h],
                        v_store[:, okt, h, :],
                        opt[h],
                        start=(okt == 0), stop=(okt == NKT - 1),
                    )

            def emit_oproj(qc, pool):
                # concatenated out-proj: one K=128 matmul per q-tile sums
                # both heads' normalized contributions in PSUM
                osb = outp.tile([P, 4, C], BF16, tag="osb", name="osb")
                for i in range(4):
                    qt = 4 * qc + i
                    ps = pool.tile([P, C], F32, tag="o_ps")
                    nc.tensor.matmul(
                        ps, attn[:, qt * P:(qt + 1) * P],
                        wo_t, start=True, stop=True)
                    if i % 2 == 0:
                        nc.scalar.copy(osb[:, i, :], ps)
                    else:
                        nc.vector.tensor_copy(osb[:, i, :], ps)
                nc.sync.dma_start(out[:, 4 * qc:4 * qc + 4, :], osb)

            def emit_drain(qc, att, h):
                # normalize on device: stage the denominator row to SBUF
                # (ScalarE), approx reciprocal (~18 bits; dens are
                # well-conditioned positive sums -- the custom-DVE op needs
                # SBUF input), fan out across 64 partitions (GpSimd), then
                # one STT into the concatenated attn tile
                qsl = slice(qc * QCHUNK, (qc + 1) * QCHUNK)
                rw = small.tile([1, QCHUNK], F32, tag=f"rw{h}",
                                name=f"rw{h}")
                rd = small.tile([1, QCHUNK], F32, tag=f"rd{h}",
                                name=f"rd{h}")
                rb = small.tile([DH, QCHUNK], F32, tag=f"rb{h}",
                                name=f"rb{h}")
                nc.scalar.copy(rw, att[h][DH:DH + 1, :])
                nc.vector.reciprocal_approx_fast(rd, rw)
                nc.gpsimd.partition_broadcast(rb, rd)
                nc.vector.scalar_tensor_tensor(
                    attn[h * DH:(h + 1) * DH, qsl],
                    att[h][0:DH, :], 1.0, rb, MULT, MULT)

            prev = None  # att tiles of the previous q-chunk
            for qc in range(NQC):
                qsl = slice(qc * QCHUNK, (qc + 1) * QCHUNK)
                att = [a_ps.tile([DH + 1, QCHUNK], F32, tag="att",
                                 name=f"att{_h}")
                       for _h in range(HEADS_PER_CORE)]
                pend = []  # (kt, (pt0, pt1)) waiting for their AV matmuls
                for kt in range(NKT):
                    if qc == 0:
                        emit_vproj(kt)
                    sps_h = [s_ps.tile([P, QCHUNK], F32, tag=f"sh{_h}",
                                       name=f"sps{_h}")
                             for _h in range(HEADS_PER_CORE)]
                    for h in range(HEADS_PER_CORE):
                        nc.tensor.matmul(
                            sps_h[h],
                            kTz[:, h, kt * P:(kt + 1) * P],
                            qT[:, qsl],
                            start=True, stop=True,
                        )
                    pt0 = ptp.tile([P, QCHUNK], BF16, tag="pt0")
                    pt1 = ptp.tile([P, QCHUNK], BF16, tag="pt1")
                    nc.scalar.activation(pt0, sps_h[0], EXP)
                    nc.vector.tensor_scalar(
                        pt1.bitcast(I16), sps_h[1], SCH_A, SCH_B, MULT,
                        mybir.AluOpType.add)
                    pend.append((kt, (pt0, pt1)))
                    if len(pend) > 2:
                        emit_av(att, *pend.pop(0))
                    if qc > 0 and kt == 1:
                        emit_drain(qc - 1, prev, 0)
                    if qc > 0 and kt == 6:
                        emit_drain(qc - 1, prev, 1)
                    if qc > 0 and kt == 9:
                        emit_oproj(qc - 1, o_ps)
                for p_ in pend:
                    emit_av(att, *p_)
                prev = att
            emit_drain(NQC - 1, prev, 0)
            emit_drain(NQC - 1, prev, 1)
            v_ps_cm.__exit__(None, None, None)
            o_ps_cm.__exit__(None, None, None)
            a_ps_cm.__exit__(None, None, None)
            s_ps_cm.__exit__(None, None, None)

            # final q-chunk's out-proj with a deep PSUM ring (all banks
            # free now) so its 4 matmuls don't serialize on the copies
            with tc.tile_pool(name="f_ps", bufs=4, space="PSUM") as f_ps:
                emit_oproj(NQC - 1, f_ps)

    nc.compile()
    return nc


def _get_nc(reps=1):
    key = f"nc{reps}"
    if key not in _cached:
        _cached[key] = _build(reps)
    return _cached[key]


def _build_in_maps(inputs):
    x = np.asarray(inputs["x"], dtype=np.float32)
    Wq = np.asarray(inputs["Wq"], dtype=np.float32)
    Wk = np.asarray(inputs["Wk"], dtype=np.float32)
    Wv = np.asarray(inputs["Wv"], dtype=np.float32)
    Wo = np.asarray(inputs["Wo"], dtype=np.float32)

    scale = np.float32(1.0 / np.sqrt(DH))
    in_maps = []
    for core in range(NCORES):
        b = core // CORES_PER_BATCH
        j = core % CORES_PER_BATCH
        csl = slice(j * P, (j + 1) * P)
        bf = ml_dtypes.bfloat16
        # xT [p, g, c, n_inner]: x[b].T is [C, L]; block C into (c, p) and
        # L into (g, n)
        xTb = (x[b].T.astype(bf)
               .reshape(NCC, P, NG, L // NG).transpose(1, 2, 0, 3))
        in_maps.append({
            "xT": np.ascontiguousarray(xTb),
            "wq": np.ascontiguousarray((Wq[:, csl] * scale).astype(bf)
                                       .reshape(NCC, P, P).transpose(1, 0, 2)),
            "wk": np.ascontiguousarray(Wk[:, csl].astype(bf)
                                       .reshape(NCC, P, P).transpose(1, 0, 2)),
            "wv": np.ascontiguousarray(Wv[:, csl].astype(bf)
                                       .reshape(NCC, P, P).transpose(1, 0, 2)),
            "wo": np.ascontiguousarray(Wo[csl, :].astype(bf)),
        })
    return in_maps


def kernel(x, Wq, Wk, Wv, Wo, bo):
    from concourse import bass_utils

    bo = np.asarray(bo, dtype=np.float32)
    in_maps = _build_in_maps(
        {"x": x, "Wq": Wq, "Wk": Wk, "Wv": Wv, "Wo": Wo})

    res = bass_utils.run_bass_kernel_spmd(
        _get_nc(), in_maps, core_ids=list(range(NCORES)))

    out = np.zeros((B, L, C), dtype=np.float32)
    for core in range(NCORES):
        r = res.results[core]
        b = core // CORES_PER_BATCH
        o = np.asarray(r["out"]).astype(np.float32)
        out[b] += o.transpose(1, 0, 2).reshape(L, C)
    out += bo[None, None, :]
    return out
